# revision 1
# baseline (speedup 1.0000x reference)
"""Trainium2 Bass kernel for nn_CustomCNN (LeNet-style CNN, batch 8192).

Strategy (pure data parallel over 8 cores, 1024 images each, 8 blocks of 128):
- x loaded batch-major [128 imgs (partitions), 3072 feats] -> perfect HBM bursts,
  host pre-transposes each core's slice to feature-major [3072, 1024].
- conv1 via batch-in-M matmuls: out[128 imgs, 336 outs] accumulated over 6
  chunks (3 channels x 2 row-halves); W are prebuilt sparse conv matrices in
  f32r (1 cycle/row on PE at N>=256, exact fp32 numerics).
- Software pipelining: conv1 matmuls of block b+1 are emitted BEFORE the tail
  (conv2..fc2) of block b, so the PE queue never stalls on the ACT/DVE chain.
- tanh1/tanh2 write bf16 (pool averaging washes the quantization out;
  measured end-to-end rel err ~5e-3 vs 2e-2 budget). All pool arithmetic
  carries f32 outputs. pool2 runs on the (otherwise idle) GPSIMD engine.
- The bugged avgpool (channel-mean + 2x2) collapses conv2/conv3 to
  single-channel kernels; pool scale factors folded into next-layer weights.
- Tail is feature-major and batched over groups of 4 blocks: conv3 and fc1
  run as single N=512 matmuls per group (1 cyc/row), biases via a ones-row
  (memset), fc2 uses the activation tile as the stationary operand to come
  back to image-major with no extra transpose.
"""

import sys

import numpy as np

if "/opt/trn_rl_repo" not in sys.path:
    sys.path.insert(0, "/opt/trn_rl_repo")

NCORES = 8
BPC = 1024          # images per core
NBLK = 8            # blocks of 128 images per core
P = 128

_CACHE = {}


def _build_weight_mats(k1, k2, k3, W1, b1, W2, b2):
    """Host-side construction of the dense matmul operand matrices."""
    f32 = np.float32
    k1 = np.asarray(k1, f32)
    k2e = (np.asarray(k2, f32).sum(1) / 24.0).astype(f32)   # [16,5,5] pool1 scale folded
    k3e = (np.asarray(k3, f32).sum(1) / 64.0).astype(f32)   # [120,5,5] pool2 scale folded

    # conv1: W1m[c, d, half, row=(rt*32+w), col=(ocl*112 + ohl*28 + ow)]
    # chunk rows are 4-row groups of one channel; d=0 -> rows 4a..4a+3, d=1 -> 4a+4..4a+7
    W1m = np.zeros((3, 2, 2, 128, 336), f32)
    for c in range(3):
        for d in range(2):
            for half in range(2):
                for ocl in range(3):
                    oc = half * 3 + ocl
                    for ohl in range(4):
                        for ow in range(28):
                            col = ocl * 112 + ohl * 28 + ow
                            for rt in range(4):
                                kh = rt + 4 * d - ohl
                                if 0 <= kh < 5:
                                    for kw in range(5):
                                        W1m[c, d, half, rt * 32 + ow + kw, col] = k1[oc, c, kh, kw]

    # conv2 (collapsed): input s1 [14,14]; chunk = 9 rows x 14 cols = 126 feats.
    # W2m[row=(rt*14+w), col=(oc*50 + ohl*10 + ow)] ; rows 126/127 zero-padded.
    W2m = np.zeros((128, 800), f32)
    for oc in range(16):
        for ohl in range(5):
            for ow in range(10):
                col = oc * 50 + ohl * 10 + ow
                for kh in range(5):
                    rt = ohl + kh          # 0..8
                    for kw in range(5):
                        W2m[rt * 14 + ow + kw, col] = k2e[oc, kh, kw]

    # conv3 (collapsed to matmul): s2 [25] -> 120
    K3m = np.zeros((25, 120), f32)
    for o in range(120):
        K3m[:, o] = k3e[o].reshape(25)

    return {
        "w1m": W1m,
        "w2m": W2m,
        "k3m": K3m,
        "fc1": np.asarray(W1, f32),                                  # [120, 84]
        "b1c": np.asarray(b1, f32).reshape(84, 1),                   # [84, 1]
        "fc2": np.asarray(W2, f32),                                  # [84, 10]
        "b2r": np.tile(np.asarray(b2, f32).reshape(1, 10), (128, 1)),  # [128, 10]
        "ident": np.eye(128, dtype=f32),
    }


def _build_bass(n_blocks=NBLK, n_reps=1):
    import concourse.bass as bass
    import concourse.bacc as bacc
    import concourse.mybir as mybir
    import concourse.tile as tile

    f32 = mybir.dt.float32
    f32r = mybir.dt.float32r
    bf16 = mybir.dt.bfloat16
    TANH = mybir.ActivationFunctionType.Tanh
    MS = bass.MemorySpace

    nc = bacc.Bacc("TRN2", target_bir_lowering=False, debug=False,
                   num_devices=NCORES)

    bpc = n_blocks * P
    x_d = nc.dram_tensor("x", [3072, bpc], f32r, kind="ExternalInput")
    w1_d = nc.dram_tensor("w1m", [3, 2, 2, 128, 336], f32r, kind="ExternalInput")
    w2_d = nc.dram_tensor("w2m", [128, 800], f32r, kind="ExternalInput")
    k3_d = nc.dram_tensor("k3m", [25, 120], f32r, kind="ExternalInput")
    fc1_d = nc.dram_tensor("fc1", [120, 84], f32r, kind="ExternalInput")
    b1_d = nc.dram_tensor("b1c", [84, 1], f32, kind="ExternalInput")
    fc2_d = nc.dram_tensor("fc2", [84, 10], f32r, kind="ExternalInput")
    b2_d = nc.dram_tensor("b2r", [128, 10], f32, kind="ExternalInput")
    id_d = nc.dram_tensor("ident", [128, 128], f32r, kind="ExternalInput")
    out_d = nc.dram_tensor("out", [bpc, 10], f32, kind="ExternalOutput")

    nvb = n_blocks * n_reps

    with tile.TileContext(nc) as tc:
        with (
            tc.tile_pool(name="consts", bufs=1) as consts,
            tc.tile_pool(name="chk", bufs=2) as chk,
            tc.tile_pool(name="t1p", bufs=2) as t1p,
            tc.tile_pool(name="tmp1", bufs=2) as tmp1,
            tc.tile_pool(name="s1p", bufs=2) as s1p,
            tc.tile_pool(name="s1Tp", bufs=2) as s1Tp,
            tc.tile_pool(name="t2p", bufs=2) as t2p,
            tc.tile_pool(name="tmp2", bufs=2) as tmp2,
            tc.tile_pool(name="s2ap", bufs=2) as s2ap,
            tc.tile_pool(name="s2Tp", bufs=2) as s2Tp,
            tc.tile_pool(name="t3p", bufs=2) as t3p,
            tc.tile_pool(name="t4p", bufs=2) as t4p,
            tc.tile_pool(name="outp", bufs=1) as outp,
            tc.tile_pool(name="ps1", bufs=2, space=MS.PSUM) as ps1p,
            tc.tile_pool(name="pss", bufs=4, space=MS.PSUM) as pss,
        ):
            # ---- constants into SBUF (once) ----
            w1sb = consts.tile([128, 12 * 336], f32r, tag="w1sb")
            for c in range(3):
                for d in range(2):
                    for h in range(2):
                        k = (c * 2 + d) * 2 + h
                        nc.sync.dma_start(w1sb[:, k * 336:(k + 1) * 336], w1_d[c, d, h])
            w2sb = consts.tile([128, 800], f32r, tag="w2sb")
            nc.sync.dma_start(w2sb[:], w2_d[:])
            k3sb = consts.tile([128, 120], f32r, tag="k3sb")
            nc.sync.dma_start(k3sb[0:25, :], k3_d[:])
            fc1sb = consts.tile([128, 84], f32r, tag="fc1sb")
            nc.sync.dma_start(fc1sb[0:120, :], fc1_d[:])
            b1sb = consts.tile([128, 1], f32, tag="b1sb")
            nc.sync.dma_start(b1sb[0:84, :], b1_d[:])
            fc2sb = consts.tile([128, 10], f32r, tag="fc2sb")
            nc.sync.dma_start(fc2sb[0:84, :], fc2_d[:])
            b2sb = consts.tile([128, 10], f32, tag="b2sb")
            nc.sync.dma_start(b2sb[:], b2_d[:])
            ident = consts.tile([128, 128], f32r, tag="ident")
            nc.sync.dma_start(ident[:], id_d[:])
            out_sb = outp.tile([128, n_blocks * 10], f32, tag="outsb")

            def w1t(c, d, h):
                k = (c * 2 + d) * 2 + h
                return w1sb[:, k * 336:(k + 1) * 336]

            xr = x_d[:].rearrange("(k p) n -> p k n", p=128)

            s1h = {}            # vb -> s1 tile
            grp = {}            # g -> {"s2a": tile, "members": [vb...]}

            def emit_conv1(vb):
                b = vb % n_blocks
                chunks = chk.tile([128, 3072], f32r, tag="chunks")
                c3 = chunks[:].rearrange("p (k i) -> p k i", i=128)
                # x loads go on the ACT engine's DMA ring so they overlap the
                # const loads issued on the SP ring at startup.
                nc.scalar.dma_start(c3, xr[:, :, b * P:(b + 1) * P])

                def chunk(c, g):   # channel c, 4-row group g (0..7)
                    k = c * 8 + g
                    return chunks[:, k * 128:(k + 1) * 128]

                # ---- conv1 + tanh1 -> t1 (bf16) [128, 6*28*28] (oc, oh, ow) ----
                t1 = t1p.tile([128, 4704], bf16, tag="t1")
                t1h = t1[:].rearrange("p (h ocl oh ow) -> p h ocl oh ow",
                                      ocl=3, oh=28, ow=28)

                def finish_a(a, ps):
                    dst = t1h[:, :, :, 4 * a:4 * a + 4, :]
                    srcap = ps[:].rearrange("p (h x) -> p h x", h=2)[:, :, 0:336]
                    srcap = srcap.rearrange("p h (ocl oh ow) -> p h ocl oh ow",
                                            oh=4, ow=28)
                    nc.scalar.activation(dst, srcap, TANH)

                # Chunk-major order: each chunk (c, g) is the stationary
                # operand of 4 consecutive matmuls (2 halves x a-groups g-1,
                # g), so LDWEIGHTS is amortized 4x. Two PSUM accumulation
                # groups are open at any time (ps1p bufs=2).
                psa = {}
                for g in range(8):
                    if g < 7:
                        psa[g] = ps1p.tile([128, 1024], f32, tag="ps1",
                                           name="ps1")
                    for c in range(3):
                        ck = chunk(c, g)
                        if g < 7:      # d=0 contribution to a-group g
                            for h in range(2):
                                nc.tensor.matmul(
                                    psa[g][:, h * 512:h * 512 + 336],
                                    ck, w1t(c, 0, h), start=(c == 0), stop=False)
                        if g >= 1:     # d=1 contribution to a-group g-1
                            for h in range(2):
                                nc.tensor.matmul(
                                    psa[g - 1][:, h * 512:h * 512 + 336],
                                    ck, w1t(c, 1, h), start=False, stop=(c == 2))
                    if g >= 1:
                        finish_a(g - 1, psa.pop(g - 1))

                # ---- pool1: sum 6 channels (tree) + 2x2 sum; f32 outputs ----
                u1 = tmp1.tile([128, 2352], f32, tag="u1")
                nc.vector.tensor_add(u1[:], t1[:, 0:2352], t1[:, 2352:4704])
                u = tmp1.tile([128, 784], f32, tag="u")
                nc.vector.tensor_add(u[:], u1[:, 0:784], u1[:, 784:1568])
                nc.vector.tensor_add(u[:], u[:], u1[:, 1568:2352])
                ur = u[:].rearrange("p (i t w) -> p t i w", t=2, w=28)  # i=14
                v = tmp1.tile([128, 392], f32, tag="v")                 # [14, 28]
                vr = v[:].rearrange("p (i w) -> p i w", w=28)
                nc.vector.tensor_add(vr, ur[:, 0], ur[:, 1])
                v2 = v[:].rearrange("p (i j t) -> p t i j", t=2, j=14)
                s1 = s1p.tile([128, 196], f32r, tag="s1")               # [14, 14]
                s1r = s1[:].rearrange("p (i j) -> p i j", j=14)
                nc.vector.tensor_add(s1r, v2[:, 0], v2[:, 1])
                s1h[vb] = s1

            def emit_tailA(vb):
                s1 = s1h.pop(vb)
                g = vb // 4
                if g not in grp:
                    grp[g] = {"s2a": s2ap.tile([128, 128], f32r, tag="s2a",
                                               name="s2a"),
                              "members": []}
                grp[g]["members"].append(vb)
                m = len(grp[g]["members"]) - 1
                s2a = grp[g]["s2a"]

                # ---- conv2 + tanh2 -> t2 (bf16) [128, 16*10*10] ----
                t2 = t2p.tile([128, 1600], bf16, tag="t2")
                t2r = t2[:].rearrange("p (oc oh ow) -> p oc oh ow", oh=10, ow=10)
                s1Ts = []
                for ch in range(2):
                    ptc = pss.tile([128, 512], f32r, tag="pss")
                    nc.tensor.transpose(ptc[0:126, 0:128], s1[:, ch * 70:ch * 70 + 126], ident[:])
                    s1T = s1Tp.tile([128, 128], f32r, tag=f"s1T{ch}")
                    nc.vector.tensor_copy(s1T[0:126, :], ptc[0:126, 0:128])
                    s1Ts.append(s1T)
                for ch in range(2):
                    for h2 in range(2):
                        ps2 = pss.tile([128, 512], f32, tag="pss")
                        nc.tensor.matmul(ps2[:, 0:400], s1Ts[ch][0:126, :],
                                         w2sb[0:126, h2 * 400:(h2 + 1) * 400])
                        dst = t2r[:, 8 * h2:8 * h2 + 8, 5 * ch:5 * ch + 5, :]
                        srcap = ps2[:, 0:400].rearrange(
                            "p (ocl oh ow) -> p ocl oh ow", oh=5, ow=10)
                        nc.scalar.activation(dst, srcap, TANH)

                # ---- pool2 on GPSIMD: sum 16 channels (tree) + 2x2 ----
                a2 = tmp2.tile([128, 800], f32, tag="a2")
                nc.gpsimd.tensor_add(a2[:], t2[:, 0:800], t2[:, 800:1600])
                b2t = tmp2.tile([128, 400], f32, tag="b2t")
                nc.gpsimd.tensor_add(b2t[:], a2[:, 0:400], a2[:, 400:800])
                c2t = tmp2.tile([128, 200], f32, tag="c2t")
                nc.gpsimd.tensor_add(c2t[:], b2t[:, 0:200], b2t[:, 200:400])
                d2t = tmp2.tile([128, 100], f32, tag="d2t")
                nc.gpsimd.tensor_add(d2t[:], c2t[:, 0:100], c2t[:, 100:200])
                d2r = d2t[:].rearrange("p (i t w) -> p t i w", t=2, w=10)  # i=5
                e2 = tmp2.tile([128, 50], f32, tag="e2")                   # [5, 10]
                e2r = e2[:].rearrange("p (i w) -> p i w", w=10)
                nc.gpsimd.tensor_add(e2r, d2r[:, 0], d2r[:, 1])
                e2v = e2[:].rearrange("p (i j t) -> p t i j", t=2, j=5)
                s2r = s2a[:, 32 * m:32 * m + 25].rearrange("p (i j) -> p i j", j=5)
                nc.gpsimd.tensor_add(s2r, e2v[:, 0], e2v[:, 1])

            def emit_tailB(g):
                members = grp[g]["members"]
                s2a = grp[g]["s2a"]
                L = len(members)

                # ---- batched transpose s2 -> s2T [25, 128*L] ----
                ptT = pss.tile([128, 512], f32r, tag="pss")
                for m in range(L):
                    nc.tensor.transpose(ptT[0:25, 128 * m:128 * (m + 1)],
                                        s2a[:, 32 * m:32 * m + 25], ident[:])
                s2T = s2Tp.tile([128, 512], f32r, tag="s2T")
                nc.vector.tensor_copy(s2T[0:25, 0:128 * L], ptT[0:25, 0:128 * L])

                # ---- conv3 (25->120) + tanh3, feature-major ----
                ps3 = pss.tile([128, 512], f32, tag="pss")
                nc.tensor.matmul(ps3[0:120, 0:128 * L], k3sb[0:25, 0:120],
                                 s2T[0:25, 0:128 * L])
                t3a = t3p.tile([128, 512], f32r, tag="t3a")
                nc.scalar.activation(t3a[0:120, 0:128 * L], ps3[0:120, 0:128 * L], TANH)

                # ---- fc1 + tanh4 (bias via per-partition activation bias) ----
                ps4 = pss.tile([128, 512], f32, tag="pss")
                nc.tensor.matmul(ps4[0:84, 0:128 * L], fc1sb[0:120, 0:84],
                                 t3a[0:120, 0:128 * L])
                t4a = t4p.tile([128, 512], f32r, tag="t4a")
                nc.scalar.activation(t4a[0:84, 0:128 * L], ps4[0:84, 0:128 * L], TANH,
                                     bias=b1sb[0:84, 0:1])

                # ---- fc2: activation tile as stationary -> image-major out ----
                ps5 = pss.tile([128, 512], f32, tag="pss")
                for m, vb in enumerate(members):
                    nc.tensor.matmul(ps5[:, 10 * m:10 * (m + 1)],
                                     t4a[0:84, 128 * m:128 * (m + 1)],
                                     fc2sb[0:84, :])
                for m, vb in enumerate(members):
                    b = vb % n_blocks
                    nc.vector.tensor_add(out_sb[:, b * 10:(b + 1) * 10],
                                         ps5[:, 10 * m:10 * (m + 1)], b2sb[:])

            # ---- software-pipelined emission ----
            seq = list(range(nvb))
            done_tailB = set()
            for i, vb in enumerate(seq):
                emit_conv1(vb)
                if i >= 1:
                    j = seq[i - 1]
                    emit_tailA(j)
                    if j % 4 == 0 and j >= 4 and (j // 4 - 1) not in done_tailB:
                        emit_tailB(j // 4 - 1)
                        done_tailB.add(j // 4 - 1)
            emit_tailA(seq[-1])
            for g in sorted(grp):
                if g not in done_tailB:
                    emit_tailB(g)
                    done_tailB.add(g)

            # ---- one output DMA: SBUF [128, nblk*10] -> DRAM [nblk*128, 10] ----
            od = out_d[:].rearrange("(blk p) f -> p blk f", p=P)
            ob = out_sb[:].rearrange("p (blk f) -> p blk f", f=10)
            nc.sync.dma_start(od, ob)

    nc.compile()
    return nc


def _get_nc(n_blocks=NBLK, n_reps=1):
    key = ("nc", n_blocks, n_reps)
    if key not in _CACHE:
        _CACHE[key] = _build_bass(n_blocks, n_reps)
    return _CACHE[key]


def kernel(n_reps=1, **inputs):
    x = np.asarray(inputs["x"], np.float32)
    wm = _build_weight_mats(inputs["k1"], inputs["k2"], inputs["k3"],
                            inputs["W1"], inputs["b1"], inputs["W2"], inputs["b2"])
    nc = _get_nc(NBLK, n_reps)

    from concourse.bass_utils import run_bass_kernel_spmd

    in_maps = []
    for core in range(NCORES):
        xc = np.ascontiguousarray(
            x[core * BPC:(core + 1) * BPC].reshape(BPC, 3072).T)
        m = {"x": xc}
        m.update(wm)
        in_maps.append(m)

    res = run_bass_kernel_spmd(nc, in_maps, core_ids=list(range(NCORES)))
    _CACHE["last_result"] = res
    out = np.concatenate([r["out"] for r in res.results], axis=0)
    return out.astype(np.float32)

